# revision 19
# baseline (speedup 1.0000x reference)
"""MemTransformerLM (Transformer-XL) forward on 8 Trainium2 NeuronCores.

Sharding: data-parallel over batch (4) x query-split (2): each core computes
all 12 heads / full FFN for its 256 query tokens. The only collective is one
bf16 AllGather of h per layer, fully hidden behind the next layer's q/BD
phase (BD depends only on rkT, which is host-precomputed, and q, which reads
the core-local h half).

Numerics: bf16 storage / fp32 PSUM accumulation; LayerNorm statistics in
fp32/fp32r. rel_shift via DRAM skew round-trip (write row stride SKW=1567,
read row stride SKW-1) with sentinel prefill providing causal masking; the
per-core read-base (511-256*half) is a runtime register so one SPMD module
serves both halves. prob transposed j-major via the DMA XBAR transpose
(one [128,1152] transpose per (head, q-tile)). K=64 score matmuls and M=64
AV matmuls are issued in adjacent pairs so the PE packs them 2x via implicit
tile_position.

Self-contained: hardcodes all shapes; takes full inputs, returns full output.
"""
import os
import sys
import types

sys.path.insert(0, '/opt/trn_rl_repo')

import numpy as np


def _install_axon_ntff_shim():
    try:
        from antenv import axon_hooks  # noqa: F401
        return
    except ImportError:
        pass
    try:
        import antenv
        mod = types.ModuleType("antenv.axon_hooks")
        mod._hook = None

        def _set(h):
            mod._hook = h

        def _get():
            return mod._hook

        mod.set_axon_ntff_profile_hook = _set
        mod.get_axon_ntff_profile_hook = _get
        sys.modules["antenv.axon_hooks"] = mod
        antenv.axon_hooks = mod
        from trn_agent_boot.trn_boot import _ntff_profile_via_ctypes
        hook = _ntff_profile_via_ctypes('/opt/axon/libaxon_pjrt.so')
        if hook is not None:
            mod.set_axon_ntff_profile_hook(hook)
    except Exception:
        pass


_install_axon_ntff_shim()

import concourse.bass as bass
import concourse.mybir as mybir
import concourse.tile as tile
from concourse import bacc
from concourse.bass_utils import run_bass_kernel_spmd

F32 = mybir.dt.float32
F32R = mybir.dt.float32r
BF16 = mybir.dt.bfloat16

L_FULL, NH, DH, D, DI, V = 12, 12, 64, 768, 3072, 10000
QLEN, MLEN, CLEN, BSZ = 512, 512, 32, 4
KLEN = CLEN + MLEN + QLEN           # 1056
QL = 256                            # local queries per core
EPS = 1e-5
N_CORES = 8
SCALE = 1.0 / float(np.sqrt(DH))
SENT = -1.0e38
SKW = KLEN + QLEN - 1               # 1567
BLK = 256 * SKW + 4096              # per-head skew block (elements)
RWIN = 128 * (SKW - 1)              # dynamic read window per q-tile
PADW = 1152                         # prob padded width (9*128)
P = 128
PAIRS = 6
KCH = [(0, 512), (512, 512), (1024, 32)]
# worst-case (half=1 / global tiles 2,3) chunkings; half=0's extra cols are
# absorbed by the sentinel.
BDCH = {0: [(128, 384), (512, 512), (1024, 32)],
        1: [(0, 512), (512, 512), (1024, 32)]}
ACCH = {0: [(0, 512), (512, 417)],
        1: [(0, 512), (512, 512), (1024, 32)]}

_BUILD_CACHE = {}


def _build(L):
    nc = bacc.Bacc("TRN2", target_bir_lowering=False, debug=False,
                   num_devices=N_CORES)

    def din(name, shape, dt):
        return nc.dram_tensor(name, shape, dt, kind="ExternalInput")

    h0T_d = din("h0T", [D, QLEN], BF16)
    h0own_d = din("h0own", [D, QL], BF16)
    condT_d = din("condT", [D, CLEN], BF16)
    memsT_d = din("memsT", [L, D, MLEN], BF16)
    rkT_d = din("rkT", [L, D, KLEN], BF16)
    wq_d = din("wq", [L, D, D], BF16)
    wk_d = din("wk", [L, D, D], BF16)
    wv_d = din("wv", [L, D, D], BF16)
    ow_d = din("ow", [L, D, D], BF16)
    w1_d = din("w1", [L, D, DI], BF16)
    b1_d = din("b1", [L, DI // P, P], F32)
    w2_d = din("w2", [L, DI, D], BF16)
    b2_d = din("b2", [L, D // P, P], F32)
    ln1g_d = din("ln1g", [L, D // P, P], F32)
    ln1b_d = din("ln1b", [L, D // P, P], F32)
    ln2g_d = din("ln2g", [L, D // P, P], F32)
    ln2b_d = din("ln2b", [L, D // P, P], F32)
    rwb_d = din("rwb", [D // P, P], F32)
    rrb_d = din("rrb", [D // P, P], F32)
    projw_d = din("projw", [D, V], BF16)
    projb_d = din("projb", [1, V], F32)
    out_d = nc.dram_tensor("logits", [QL, V], F32, kind="ExternalOutput")

    skew_d = nc.dram_tensor("skew", [NH * BLK], BF16, kind="Internal")
    cc_in = nc.dram_tensor("cc_in", [D, QL], BF16, kind="Internal")
    cc_out = nc.dram_tensor("cc_out", [2 * D, QL], BF16, kind="Internal")
    RG = [[0, 1], [2, 3], [4, 5], [6, 7]]

    with tile.TileContext(nc) as tc:
        import contextlib
        ctx = contextlib.ExitStack()
        with ctx:
            ctx.enter_context(nc.allow_low_precision("bf16 kernel by design"))
            const = ctx.enter_context(tc.tile_pool(name="const", bufs=1))
            persist = ctx.enter_context(tc.tile_pool(name="persist", bufs=1))
            lw = ctx.enter_context(tc.tile_pool(name="lw", bufs=1))
            wstream = ctx.enter_context(tc.tile_pool(name="wstream", bufs=2))
            work = ctx.enter_context(tc.tile_pool(name="work", bufs=2))
            bdip = ctx.enter_context(tc.tile_pool(name="bdip", bufs=4))
            bds = ctx.enter_context(tc.tile_pool(name="bds", bufs=4))
            probp = ctx.enter_context(tc.tile_pool(name="probp", bufs=4))
            probtp = ctx.enter_context(tc.tile_pool(name="probtp", bufs=2))
            small = ctx.enter_context(tc.tile_pool(name="small", bufs=1))
            smalls = ctx.enter_context(tc.tile_pool(name="smalls", bufs=4))
            ps_big = ctx.enter_context(tc.tile_pool(name="psb", bufs=2, space="PSUM"))
            ps_sm = ctx.enter_context(tc.tile_pool(name="pss", bufs=2, space="PSUM"))

            # ---- constants ----
            from concourse.masks import make_identity
            ident_f = const.tile([P, P], F32)
            make_identity(nc, ident_f)
            ident_b = const.tile([P, P], BF16)
            nc.vector.tensor_copy(out=ident_b, in_=ident_f)
            ones_f = const.tile([P, 1], F32)
            nc.vector.memset(ones_f, 1.0)
            ones_b = const.tile([P, 1], BF16)
            nc.vector.tensor_copy(out=ones_b, in_=ones_f)
            ones_r = const.tile([P, 1], F32R)
            nc.vector.tensor_copy(out=ones_r, in_=ones_f)
            onesrow_f = const.tile([1, P], F32)
            nc.vector.memset(onesrow_f, 1.0)
            onesrow_r = const.tile([1, P], F32R)
            nc.vector.tensor_copy(out=onesrow_r, in_=onesrow_f)
            sent_t = const.tile([P, SKW], BF16)
            nc.vector.memset(sent_t, SENT)
            rwb_t = const.tile([P, 6], F32)
            nc.sync.dma_start(out=rwb_t, in_=rwb_d.ap().rearrange("k p -> p k"))
            rrb_t = const.tile([P, 6], F32)
            nc.sync.dma_start(out=rrb_t, in_=rrb_d.ap().rearrange("k p -> p k"))
            eps_t = const.tile([1, 1], F32)
            nc.vector.memset(eps_t, EPS)

            # per-core dynamic skew read base: 511 - 256*(core % 2)
            pid = nc.scalar.partition_id()
            roff = nc.scalar.snap(511 - (pid % 2) * 256, min_val=255, max_val=511)

            # ---- persistent activations ----
            cat = persist.tile([P, 6, KLEN], BF16)       # cond | mems | h
            h_own = persist.tile([P, 6, QL], BF16)       # own-half h (d-major)
            h1 = persist.tile([P, 6, QL], BF16)
            hln = persist.tile([P, 6, QL], BF16)
            h2 = persist.tile([P, 6, QL], BF16)
            kT = persist.tile([P, 6, KLEN], BF16)
            qrw = persist.tile([P, 6, QL], BF16)
            qrr = persist.tile([P, 6, QL], BF16)
            v_tok = persist.tile([P, 9, D], BF16)
            av_sb = persist.tile([P, 6, QL], BF16)
            ffn1 = persist.tile([P, DI // P, QL], BF16)

            nc.vector.memset(v_tok, 0.0)

            # ---- init: cat <- cond | (mems: per layer) | h0; h_own <- own h0
            nc.sync.dma_start(out=cat[:, :, 0:CLEN],
                              in_=condT_d.ap().rearrange("(k p) t -> p k t", p=P))
            nc.sync.dma_start(out=cat[:, :, CLEN + MLEN:KLEN],
                              in_=h0T_d.ap().rearrange("(k p) t -> p k t", p=P))
            nc.sync.dma_start(out=h_own,
                              in_=h0own_d.ap().rearrange("(k p) t -> p k t", p=P))

            # ---- skew sentinel prefill ----
            for n in range(NH):
                for t in range(2):
                    dst = bass.AP(tensor=skew_d.ap().tensor,
                                  offset=n * BLK + t * 128 * SKW,
                                  ap=[[SKW, 128], [1, SKW]])
                    nc.gpsimd.dma_start(out=dst, in_=sent_t)

            def ln_dmajor(src_t, g_sb, b_sb, out_t):
                """LayerNorm over D for d-major [128, 6, QL] bf16 src."""
                s1 = ps_sm.tile([1, QL], F32, tag="sm")
                for k in range(6):
                    nc.tensor.matmul(s1, ones_b, src_t[:, k, :],
                                     start=(k == 0), stop=(k == 5))
                s2 = ps_sm.tile([1, QL], F32, tag="sm")
                for k in range(6):
                    sq = work.tile([P, QL], F32R, tag="lnsq")
                    nc.vector.tensor_mul(out=sq, in0=src_t[:, k, :],
                                         in1=src_t[:, k, :])
                    nc.tensor.matmul(s2, ones_r, sq,
                                     start=(k == 0), stop=(k == 5))
                mean = small.tile([1, QL], F32R, tag="mean")
                nc.scalar.mul(out=mean, in_=s1, mul=1.0 / D)
                msq = small.tile([1, QL], F32, tag="msq")
                nc.vector.tensor_mul(out=msq, in0=mean, in1=mean)
                var = small.tile([1, QL], F32, tag="var")
                nc.scalar.mul(out=var, in_=s2, mul=1.0 / D)
                nc.vector.tensor_sub(out=var, in0=var, in1=msq)
                nc.scalar.activation(out=var, in_=var,
                                     func=mybir.ActivationFunctionType.Sqrt,
                                     bias=eps_t, scale=1.0)
                rstd = small.tile([1, QL], F32R, tag="rstd")
                nc.vector.reciprocal(out=rstd, in_=var)
                meanB = ps_sm.tile([P, QL], F32, tag="sm")
                nc.tensor.matmul(meanB, onesrow_r, mean, start=True, stop=True)
                rstdB = ps_sm.tile([P, QL], F32, tag="sm")
                nc.tensor.matmul(rstdB, onesrow_r, rstd, start=True, stop=True)
                for k in range(6):
                    tmp = work.tile([P, QL], F32, tag="lnt")
                    nc.vector.tensor_sub(out=tmp, in0=src_t[:, k, :], in1=meanB)
                    nc.vector.tensor_mul(out=tmp, in0=tmp, in1=rstdB)
                    nc.vector.tensor_scalar(out=out_t[:, k, :], in0=tmp,
                                            scalar1=g_sb[:, k:k+1],
                                            scalar2=b_sb[:, k:k+1],
                                            op0=mybir.AluOpType.mult,
                                            op1=mybir.AluOpType.add)

            # ============================ layers ============================
            for l in range(L):
                # ---- layer weights ----
                wq_sb = lw.tile([P, 6, D], BF16, tag="wq")
                nc.sync.dma_start(out=wq_sb, in_=wq_d.ap()[l].rearrange("(k p) m -> p k m", p=P))
                wk_sb = lw.tile([P, 6, D], BF16, tag="wk")
                nc.sync.dma_start(out=wk_sb, in_=wk_d.ap()[l].rearrange("(k p) m -> p k m", p=P))
                wv_sb = lw.tile([P, 6, D], BF16, tag="wv")
                nc.sync.dma_start(out=wv_sb, in_=wv_d.ap()[l].rearrange("(k p) m -> p k m", p=P))
                ow_sb = lw.tile([P, 6, D], BF16, tag="ow")
                nc.sync.dma_start(out=ow_sb, in_=ow_d.ap()[l].rearrange("(k p) m -> p k m", p=P))
                rkT_sb = lw.tile([P, 6, KLEN], BF16, tag="rkT")
                nc.sync.dma_start(out=rkT_sb, in_=rkT_d.ap()[l].rearrange("(k p) t -> p k t", p=P))
                nc.sync.dma_start(out=cat[:, :, CLEN:CLEN + MLEN],
                                  in_=memsT_d.ap()[l].rearrange("(k p) t -> p k t", p=P))
                b1_sb = lw.tile([P, DI // P], F32, tag="b1")
                nc.sync.dma_start(out=b1_sb, in_=b1_d.ap()[l].rearrange("k p -> p k"))
                b2_sb = lw.tile([P, 6], F32, tag="b2")
                nc.sync.dma_start(out=b2_sb, in_=b2_d.ap()[l].rearrange("k p -> p k"))
                ln1g_sb = lw.tile([P, 6], F32, tag="ln1g")
                nc.sync.dma_start(out=ln1g_sb, in_=ln1g_d.ap()[l].rearrange("k p -> p k"))
                ln1b_sb = lw.tile([P, 6], F32, tag="ln1b")
                nc.sync.dma_start(out=ln1b_sb, in_=ln1b_d.ap()[l].rearrange("k p -> p k"))
                ln2g_sb = lw.tile([P, 6], F32, tag="ln2g")
                nc.sync.dma_start(out=ln2g_sb, in_=ln2g_d.ap()[l].rearrange("k p -> p k"))
                ln2b_sb = lw.tile([P, 6], F32, tag="ln2b")
                nc.sync.dma_start(out=ln2b_sb, in_=ln2b_d.ap()[l].rearrange("k p -> p k"))

                # ---- q-proj + rel biases (local; overlaps the in-flight AG) ----
                for m in range(6):
                    pq = ps_sm.tile([P, QL], F32, tag="sm")
                    for k in range(6):
                        nc.tensor.matmul(pq, wq_sb[:, k, m * P:(m + 1) * P],
                                         h_own[:, k, :], start=(k == 0), stop=(k == 5))
                    nc.vector.tensor_scalar_add(out=qrw[:, m, :], in0=pq,
                                                scalar1=rwb_t[:, m:m+1])
                    nc.vector.tensor_scalar_add(out=qrr[:, m, :], in0=pq,
                                                scalar1=rrb_t[:, m:m+1])

                # ---- BD phase (also AG-overlapped): packed K=64 pairs ----
                for g in range(PAIRS):
                    for tl in range(2):
                        pbA = ps_big.tile([P, KLEN], F32, tag="big")
                        pbB = ps_big.tile([P, KLEN], F32, tag="big")
                        for (c0, w) in BDCH[tl]:
                            nc.tensor.matmul(pbA[:, c0:c0 + w],
                                             qrr[0:64, g, tl * P:(tl + 1) * P],
                                             rkT_sb[0:64, g, c0:c0 + w],
                                             start=True, stop=True)
                            nc.tensor.matmul(pbB[:, c0:c0 + w],
                                             qrr[64:128, g, tl * P:(tl + 1) * P],
                                             rkT_sb[64:128, g, c0:c0 + w],
                                             start=True, stop=True)
                        c0min = BDCH[tl][0][0]
                        bdA = bdip.tile([P, KLEN], BF16, tag="bdi")
                        nc.vector.tensor_copy(out=bdA[:, c0min:], in_=pbA[:, c0min:])
                        bdB = bdip.tile([P, KLEN], BF16, tag="bdi")
                        nc.scalar.copy(out=bdB[:, c0min:], in_=pbB[:, c0min:])
                        for (n, bd_i) in ((2 * g, bdA), (2 * g + 1, bdB)):
                            dst = bass.AP(tensor=skew_d.ap().tensor,
                                          offset=n * BLK + tl * 128 * SKW + c0min,
                                          ap=[[SKW, 128], [1, KLEN - c0min]])
                            nc.sync.dma_start(out=dst, in_=bd_i[:, c0min:])

                # ---- readback of previous layer's AllGather into cat ----
                if l > 0:
                    for h in range(2):
                        nc.sync.dma_start(
                            out=cat[:, :, CLEN + MLEN + h * QL:CLEN + MLEN + (h + 1) * QL],
                            in_=cc_out.ap()[h * D:(h + 1) * D].rearrange(
                                "(k p) t -> p k t", p=P))

                # ---- kT (waits for AG of previous layer via cat) ----
                for m in range(6):
                    for (c0, w) in KCH:
                        pk = ps_sm.tile([P, 512], F32, tag="sm")
                        for k in range(6):
                            nc.tensor.matmul(pk[:, 0:w], wk_sb[:, k, m * P:(m + 1) * P],
                                             cat[:, k, c0:c0 + w],
                                             start=(k == 0), stop=(k == 5))
                        nc.vector.tensor_copy(out=kT[:, m, c0:c0 + w], in_=pk[:, 0:w])

                # ---- v token-major (128-grid segments) ----
                for s in range(9):
                    off = 128 * s
                    w = 32 if s == 8 else 128
                    pv1 = ps_sm.tile([P, 512], F32, tag="sm")
                    pv2 = ps_sm.tile([P, 512], F32, tag="sm")
                    for k in range(6):
                        nc.tensor.matmul(pv1[0:w, :], cat[:, k, off:off + w],
                                         wv_sb[:, k, 0:512],
                                         start=(k == 0), stop=(k == 5))
                    for k in range(6):
                        nc.tensor.matmul(pv2[0:w, 0:256], cat[:, k, off:off + w],
                                         wv_sb[:, k, 512:768],
                                         start=(k == 0), stop=(k == 5))
                    nc.vector.tensor_copy(out=v_tok[0:w, s, 0:512], in_=pv1[0:w, :])
                    nc.vector.tensor_copy(out=v_tok[0:w, s, 512:768], in_=pv2[0:w, 0:256])

                # ---- attention: skew read + AC + softmax + transpose + AV ----
                for g in range(PAIRS):
                    probTA = probtp.tile([P, 9, QL], BF16, tag="pTA")
                    probTB = probtp.tile([P, 9, QL], BF16, tag="pTB")
                    for tl in range(2):
                        RW = 929 if tl == 0 else KLEN
                        TW = 1024 if tl == 0 else PADW
                        for (hp0, n, probT_x) in ((0, 2 * g, probTA),
                                                  (64, 2 * g + 1, probTB)):
                            bds_t = bds.tile([P, KLEN], BF16, tag="bds")
                            src = skew_d.ap()[bass.ds(
                                roff + (n * BLK + tl * 128 * (SKW - 1)),
                                RWIN)].rearrange("(p s) -> p s", p=P)[:, 0:RW]
                            nc.scalar.dma_start(out=bds_t[:, 0:RW], in_=src)
                            pa = ps_big.tile([P, KLEN], F32, tag="big")
                            for (c0, w) in ACCH[tl]:
                                nc.tensor.matmul(pa[:, c0:c0 + w], ident_b,
                                                 bds_t[:, c0:c0 + w],
                                                 start=True, stop=False)
                            nch = len(ACCH[tl])
                            for ci, (c0, w) in enumerate(ACCH[tl]):
                                nc.tensor.matmul(pa[:, c0:c0 + w],
                                                 qrw[hp0:hp0 + 64, g, tl * P:(tl + 1) * P],
                                                 kT[hp0:hp0 + 64, g, c0:c0 + w],
                                                 start=False, stop=(ci == nch - 1))
                            prob = probp.tile([P, PADW], BF16, tag="prob")
                            nc.vector.memset(prob[:, RW:TW], 0.0)
                            dnm = smalls.tile([P, 1], F32, tag="dnm")
                            nc.scalar.activation(out=prob[:, 0:RW], in_=pa[:, 0:RW],
                                                 func=mybir.ActivationFunctionType.Exp,
                                                 bias=0.0, scale=SCALE, accum_out=dnm)
                            rd = smalls.tile([P, 1], F32, tag="rd")
                            nc.vector.reciprocal(out=rd, in_=dnm)
                            nc.vector.tensor_scalar_mul(out=prob[:, 0:RW],
                                                        in0=prob[:, 0:RW], scalar1=rd)
                            if tl == 0:
                                nc.sync.dma_start(
                                    out=probT_x[:, 0:8, 0:P],
                                    in_=prob[:, 0:1024], transpose=True)
                            else:
                                nc.sync.dma_start(
                                    out=probT_x[:, :, P:2 * P],
                                    in_=prob, transpose=True)
                    pav = ps_sm.tile([P, QL], F32, tag="sm")
                    for s in range(9):
                        cl = P if s == 8 else 0
                        nc.tensor.matmul(pav[0:64, cl:QL],
                                         v_tok[:, s, 128 * g:128 * g + 64],
                                         probTA[:, s, cl:QL],
                                         start=(s == 0), stop=(s == 8))
                        nc.tensor.matmul(pav[64:128, cl:QL],
                                         v_tok[:, s, 128 * g + 64:128 * g + 128],
                                         probTB[:, s, cl:QL],
                                         start=(s == 0), stop=(s == 8))
                    nc.vector.tensor_copy(out=av_sb[:, g, :], in_=pav)

                # ---- o-proj + residual -> ln1 ----
                for m in range(6):
                    po = ps_sm.tile([P, QL], F32, tag="sm")
                    for k in range(6):
                        nc.tensor.matmul(po, ow_sb[:, k, m * P:(m + 1) * P],
                                         av_sb[:, k, :], start=(k == 0), stop=(k == 5))
                    nc.vector.tensor_add(out=hln[:, m, :], in0=po, in1=h_own[:, m, :])
                ln_dmajor(hln, ln1g_sb, ln1b_sb, h1)

                # ---- FFN ----
                for b in range(6):
                    wsl = wstream.tile([P, 6, 512], BF16, tag="w1s")
                    src = bass.AP(tensor=w1_d.ap().tensor,
                                  offset=l * D * DI + b * 512,
                                  ap=[[DI, P], [P * DI, 6], [1, 512]])
                    nc.sync.dma_start(out=wsl, in_=src)
                    for j in range(4):
                        km = 4 * b + j
                        pf = ps_sm.tile([P, QL], F32, tag="sm")
                        for k in range(6):
                            nc.tensor.matmul(pf, wsl[:, k, j * P:(j + 1) * P],
                                             h1[:, k, :], start=(k == 0), stop=(k == 5))
                        nc.scalar.activation(out=ffn1[:, km, :], in_=pf,
                                             func=mybir.ActivationFunctionType.Relu,
                                             bias=b1_sb[:, km:km+1], scale=1.0)
                for m in range(6):
                    w2a = wstream.tile([P, 12, P], BF16, tag="w2s")
                    src = bass.AP(tensor=w2_d.ap().tensor,
                                  offset=l * DI * D + m * P,
                                  ap=[[D, P], [P * D, 12], [1, P]])
                    nc.sync.dma_start(out=w2a, in_=src)
                    w2b = wstream.tile([P, 12, P], BF16, tag="w2s")
                    src = bass.AP(tensor=w2_d.ap().tensor,
                                  offset=l * DI * D + 12 * P * D + m * P,
                                  ap=[[D, P], [P * D, 12], [1, P]])
                    nc.sync.dma_start(out=w2b, in_=src)
                    pf = ps_sm.tile([P, QL], F32, tag="sm")
                    for ki in range(24):
                        wt = w2a if ki < 12 else w2b
                        nc.tensor.matmul(pf, wt[:, ki % 12, :], ffn1[:, ki, :],
                                         start=(ki == 0), stop=(ki == 23))
                    fb = work.tile([P, QL], BF16, tag="fb")
                    nc.vector.tensor_scalar_add(out=fb, in0=pf, scalar1=b2_sb[:, m:m+1])
                    nc.vector.tensor_add(out=h2[:, m, :], in0=fb, in1=h1[:, m, :])
                ln_dmajor(h2, ln2g_sb, ln2b_sb, h_own)

                # ---- AllGather h (skipped after the last layer); the
                # cat readback is emitted at the top of the next layer so it
                # does not block the sync FIFO while the AG is in flight ----
                if l < L - 1:
                    nc.gpsimd.dma_start(
                        out=cc_in.ap().rearrange("(k p) t -> p k t", p=P),
                        in_=h_own)
                    nc.gpsimd.collective_compute(
                        "AllGather", mybir.AluOpType.bypass, replica_groups=RG,
                        ins=[cc_in.ap()], outs=[cc_out.ap()])

            # ---- final projection (own 256 tokens x full vocab) ----
            NCH = 500
            for tt in range(2):
                for c in range(V // NCH):
                    pp = ps_sm.tile([P, NCH], F32, tag="sm")
                    wsl = wstream.tile([P, 6, NCH], BF16, tag="w1s")
                    src = bass.AP(tensor=projw_d.ap().tensor,
                                  offset=c * NCH,
                                  ap=[[V, P], [P * V, 6], [1, NCH]])
                    nc.sync.dma_start(out=wsl, in_=src)
                    for k in range(6):
                        nc.tensor.matmul(pp, h_own[:, k, tt * P:(tt + 1) * P],
                                         wsl[:, k, :], start=(k == 0), stop=False)
                    pbs = small.tile([1, NCH], F32R, tag="pbs")
                    nc.gpsimd.dma_start(out=pbs, in_=projb_d.ap()[:, c * NCH:(c + 1) * NCH])
                    nc.tensor.matmul(pp, onesrow_r, pbs, start=False, stop=True)
                    osb = work.tile([P, NCH], F32, tag="osb")
                    nc.vector.tensor_copy(out=osb, in_=pp)
                    nc.sync.dma_start(out=out_d.ap()[tt * P:(tt + 1) * P,
                                                     c * NCH:(c + 1) * NCH],
                                      in_=osb)

    nc.compile()
    return nc


def _pos_emb(klen):
    pos = np.arange(klen - 1, -1, -1, dtype=np.float32)
    inv = 1.0 / (10000.0 ** (np.arange(0, D, 2, dtype=np.float32) / D))
    s = pos[:, None] * inv[None, :]
    return np.concatenate([np.sin(s), np.cos(s)], axis=-1)  # [klen, D]


def kernel(x, condition, mems, emb, qkv_w, r_net_w, o_w, ln1_g, ln1_b,
           w1, b1, w2, b2, ln2_g, ln2_b, r_w_bias, r_r_bias, proj_w, proj_b):
    import ml_dtypes
    BF = ml_dtypes.bfloat16

    L = int(os.environ.get("KERNEL_LAYERS", str(L_FULL)))
    if L not in _BUILD_CACHE:
        _BUILD_CACHE[L] = _build(L)
    nc = _BUILD_CACHE[L]

    f32 = lambda a: np.asarray(a, dtype=np.float32)
    bf = lambda a: np.ascontiguousarray(np.asarray(a, dtype=np.float32).astype(BF))
    x = np.asarray(x)
    condition = f32(condition); mems = f32(mems); emb = f32(emb)
    qkv_w = f32(qkv_w); r_net_w = f32(r_net_w); o_w = f32(o_w)
    ln1_g = f32(ln1_g); ln1_b = f32(ln1_b); w1 = f32(w1); b1 = f32(b1)
    w2 = f32(w2); b2 = f32(b2); ln2_g = f32(ln2_g); ln2_b = f32(ln2_b)
    r_w_bias = f32(r_w_bias); r_r_bias = f32(r_r_bias)
    proj_w = f32(proj_w); proj_b = f32(proj_b)

    # host precompute: rel-position keys per layer, transposed d-major
    r = _pos_emb(KLEN)                                   # [KLEN, D]
    rkT = np.stack([np.ascontiguousarray((r @ r_net_w[l]).T) for l in range(L)])
    rkT = bf(rkT)                                        # [L, D, KLEN]

    # shared (core-independent) weight tensors
    shared = {
        "condTf": condition,                              # per-batch below
        "rkT": rkT,
        "wq": bf(qkv_w[:L, :, 0:D]),
        "wk": bf(qkv_w[:L, :, D:2 * D]),
        "wv": bf(qkv_w[:L, :, 2 * D:3 * D]),
        "ow": bf(o_w[:L]),
        "w1": bf(w1[:L]),
        "b1": np.ascontiguousarray(b1[:L]).reshape(L, DI // 128, 128),
        "w2": bf(w2[:L]),
        "b2": np.ascontiguousarray(b2[:L]).reshape(L, 6, 128),
        "ln1g": np.ascontiguousarray(ln1_g[:L]).reshape(L, 6, 128),
        "ln1b": np.ascontiguousarray(ln1_b[:L]).reshape(L, 6, 128),
        "ln2g": np.ascontiguousarray(ln2_g[:L]).reshape(L, 6, 128),
        "ln2b": np.ascontiguousarray(ln2_b[:L]).reshape(L, 6, 128),
        "rwb": np.ascontiguousarray(r_w_bias.reshape(NH * DH)).reshape(6, 128),
        "rrb": np.ascontiguousarray(r_r_bias.reshape(NH * DH)).reshape(6, 128),
        "projw": bf(proj_w),
        "projb": np.ascontiguousarray(proj_b).reshape(1, V),
    }

    in_maps = []
    for c in range(N_CORES):
        b, half = c // 2, c % 2
        h0 = (emb[np.asarray(x[:, b], dtype=np.int64)]
              * np.float32(np.sqrt(D)))                   # [QLEN, D]
        h0T = bf(h0.T)                                    # [D, QLEN]
        m = {
            "h0T": h0T,
            "h0own": np.ascontiguousarray(h0T[:, half * QL:(half + 1) * QL]),
            "condT": bf(condition[:, b, :].T),
            "memsT": bf(mems[:L, :, b, :].transpose(0, 2, 1)),
        }
        for k in ("rkT", "wq", "wk", "wv", "ow", "w1", "b1", "w2", "b2",
                  "ln1g", "ln1b", "ln2g", "ln2b", "rwb", "rrb",
                  "projw", "projb"):
            m[k] = shared[k]
        in_maps.append(m)

    trace = bool(int(os.environ.get("KERNEL_TRACE", "0")))
    res = run_bass_kernel_spmd(nc, in_maps, core_ids=list(range(N_CORES)),
                               trace=trace)
    kernel.last_result = res

    out = np.zeros((QLEN, BSZ, V), np.float32)
    for c in range(N_CORES):
        b, half = c // 2, c % 2
        out[half * QL:(half + 1) * QL, :, :][:, b, :] = res.results[c]["logits"]
    return out
